# revision 5
# baseline (speedup 1.0000x reference)
"""Trainium2 Bass kernel for BasicGCNSegmentation (3-layer GCN + CAM head).

Strategy (8 NeuronCores, SPMD):
  - Nodes sharded contiguously: core c owns rows [c*12500, (c+1)*12500),
    padded on host to 12544 = 98*128 per core (100352 global padded rows).
  - Per layer: each core transforms its own node slice t = (h .* out_norm) @ W
    (PE, via per-tile transpose), AllGather of t -> replicated table in DRAM,
    then per-edge row gather (dma_gather, int16 indices in 4 address buckets,
    spread over 4 SWDGE queues) + segment-sum via one-hot matmuls accumulating
    in PSUM, epilogue relu with folded degree norms (ACT).
  - deg-norm bias fold: relu(in_n*S + b) = relu(norm*(S + b*sqrt(deg_in)))
    with norm = in_n (*out_n for layers that feed the next transform), so the
    bias enters as one rank-1 matmul into PSUM and the epilogue is a single
    scaled relu.
  - Final: node-mean via ones-matmul + AllReduce; CAM = Wp @ h3^T per tile.

Host does graph restructuring only (sharding, sorting, padding, degree
counting, int16 index marshalling); all floating-point math runs on device.
"""

import os

import numpy as np

import concourse.bacc as bacc
import concourse.bass as bass
import concourse.mybir as mybir
import concourse.tile as tile
from concourse.bass_utils import run_bass_kernel_spmd

N_CORES = 8
N = 100000
E = 1600000
H = 128
C = 40
NPC = N // N_CORES          # 12500 real nodes per core
TILES = 98
TILE = 128
PADN = TILES * TILE          # 12544 padded nodes per core
GPAD = PADN * N_CORES        # 100352 padded global rows
N_BUCKETS = 4
BUCKET = GPAD // N_BUCKETS   # 25088 rows per gather-address bucket (<32768)
SG_TILES = 3                 # dst tiles per gather supergroup
N_SG = (TILES + SG_TILES - 1) // SG_TILES  # 33 (last has 2 tiles)

f32 = mybir.dt.float32
f32r = mybir.dt.float32r
i32 = mybir.dt.int32
i16 = mybir.dt.int16


def _prep_graph(src, dst):
    """Shard/sort edges; build per-core int16 gather indices + one-hot ids.

    Chunk structure (counts per (tile, bucket)) is the max over cores, so one
    SPMD program fits all cores; per-core arrays differ only in values.
    """
    src = src.astype(np.int64)
    dst = dst.astype(np.int64)
    src_pad = src + (src // NPC) * (PADN - NPC)

    core_of = dst // NPC
    dl = dst % NPC
    t_id = dl >> 7
    wid = (dl & 127).astype(np.int32)
    b_id = src_pad // BUCKET
    rel = (src_pad - b_id * BUCKET).astype(np.int16)

    key = (core_of * TILES * N_BUCKETS + t_id * N_BUCKETS + b_id).astype(np.int64)
    counts = np.bincount(key, minlength=N_CORES * TILES * N_BUCKETS).reshape(
        N_CORES, TILES, N_BUCKETS
    )
    chunks_tb = -(-counts.max(axis=0) // TILE)  # [98, 4] ceil
    empty = chunks_tb.sum(axis=1) == 0
    chunks_tb[empty, 0] = 1
    chunks_tb = chunks_tb.astype(np.int64)

    # global chunk order: (sg, b, t within sg, k); calls are (sg, b) blocks
    cg_start = np.zeros((TILES, N_BUCKETS), dtype=np.int64)
    call_plan = []  # (sg, b, n_chunks, cg_base, idx_col_off)
    cg = 0
    idx_off = 0
    for sg in range(N_SG):
        ts = range(sg * SG_TILES, min((sg + 1) * SG_TILES, TILES))
        for b in range(N_BUCKETS):
            cc = 0
            for t in ts:
                cg_start[t, b] = cg + cc
                cc += chunks_tb[t, b]
            call_plan.append((sg, b, int(cc), cg, idx_off))
            cg += cc
            idx_off += cc * 8  # int16 columns per chunk = 128/16
    nchunk = cg

    # per-tile consumption map: (bucket, slot within call); one-hot columns are
    # numbered per-tile-contiguous (sg, t, b, k), padded to x4 per tile
    call_base = {(sg, b): cgb for sg, b, cc, cgb, _ in call_plan}
    tile_chunks = []
    tile_oh_base = np.zeros(TILES, dtype=np.int64)
    ohc = 0
    for t in range(TILES):
        sg = t // SG_TILES
        tile_oh_base[t] = ohc
        lst = []
        for b in range(N_BUCKETS):
            for k in range(chunks_tb[t, b]):
                cgk = int(cg_start[t, b] + k)
                lst.append((b, cgk - call_base[(sg, b)]))
        tile_chunks.append(lst)
        ohc += (len(lst) + 3) // 4 * 4
    nchunk_oh = ohc

    idx_cores, ids_cores = [], []
    for c in range(N_CORES):
        m = core_of == c
        tc_, bc_, relc, widc = t_id[m], b_id[m], rel[m], wid[m]
        order = np.lexsort((bc_, tc_))
        tc_, bc_, relc, widc = tc_[order], bc_[order], relc[order], widc[order]
        gkey = tc_ * N_BUCKETS + bc_
        grp_counts = np.bincount(gkey, minlength=TILES * N_BUCKETS)
        grp_first = np.concatenate([[0], np.cumsum(grp_counts)[:-1]])
        within = np.arange(len(gkey)) - grp_first[gkey]
        pos = cg_start[tc_, bc_] * TILE + within

        # one-hot position: per-tile-contiguous numbering
        boff = np.zeros((TILES, N_BUCKETS), dtype=np.int64)
        for t in range(TILES):
            o = 0
            for b in range(N_BUCKETS):
                boff[t, b] = o
                o += chunks_tb[t, b]
        ohpos = (tile_oh_base[tc_] + boff[tc_, bc_]) * TILE + within

        idx_flat = np.zeros(nchunk * TILE, dtype=np.int16)
        ids_flat = np.full(nchunk_oh * TILE, -1.0, dtype=np.float32)
        idx_flat[pos] = relc
        ids_flat[ohpos] = widc

        cols = []
        for sg, b, cc, cgb, _ in call_plan:
            if cc == 0:
                continue
            blk = idx_flat[cgb * TILE : (cgb + cc) * TILE]
            cols.append(np.tile(blk.reshape(-1, 16).T, (8, 1)))
        idx_cores.append(np.ascontiguousarray(np.concatenate(cols, axis=1)))
        ids_cores.append(np.ascontiguousarray(ids_flat.reshape(nchunk_oh, TILE).T))

    return (
        chunks_tb,
        nchunk_oh,
        call_plan,
        tile_chunks,
        tile_oh_base,
        idx_cores,
        ids_cores,
    )


def kernel(features, W1, b1, W2, b2, W3, b3, Wp, bp, src, dst):
    assert features.shape == (N, H) and src.shape == (E,) and dst.shape == (E,)

    out_deg = np.bincount(src, minlength=N).astype(np.int32)
    in_deg = np.bincount(dst, minlength=N).astype(np.int32)

    (
        chunks_tb,
        NCHUNK,
        call_plan,
        tile_chunks,
        tile_oh_base,
        idx_cores,
        ids_cores,
    ) = _prep_graph(src, dst)
    IDXW = sum(cc * 8 for _, _, cc, _, _ in call_plan)

    x_cores, odeg_cores, ideg_cores = [], [], []
    for c in range(N_CORES):
        xs = np.zeros((PADN, H), dtype=np.float32)
        xs[:NPC] = np.asarray(features)[c * NPC : (c + 1) * NPC]
        x_cores.append(xs)
        od = np.zeros(PADN, dtype=np.int32)
        od[:NPC] = out_deg[c * NPC : (c + 1) * NPC]
        idg = np.zeros(PADN, dtype=np.int32)
        idg[:NPC] = in_deg[c * NPC : (c + 1) * NPC]
        odeg_cores.append(np.ascontiguousarray(od.reshape(TILES, TILE).T))
        ideg_cores.append(np.ascontiguousarray(idg.reshape(TILES, TILE).T))

    iota_np = np.ascontiguousarray(
        np.tile(np.broadcast_to(np.arange(TILE, dtype=np.float32), (TILE, TILE)), (1, 4))
    )
    ident_np = np.eye(TILE, dtype=np.float32)
    onesm_np = np.zeros((TILE, 2), dtype=np.float32)
    onesm_np[:, 0] = 1.0
    onesm_np[: NPC - 97 * TILE, 1] = 1.0
    bbc_np = np.ascontiguousarray(
        np.broadcast_to(
            np.concatenate([b1, b2, b3]).astype(np.float32), (TILE, 3 * H)
        ).copy()
    )
    ws_np = np.ascontiguousarray(np.concatenate([W1, W2, W3], axis=1).astype(np.float32))
    wpt_np = np.ascontiguousarray(np.asarray(Wp).astype(np.float32).T)
    bp_np = np.asarray(bp).astype(np.float32).reshape(C, 1)

    nc = bacc.Bacc(
        "TRN2",
        target_bir_lowering=False,
        debug=False,
        num_devices=N_CORES,
        num_swdge_queues=4,
    )

    def din(name, shape, dtype):
        return nc.dram_tensor(name, list(shape), dtype, kind="ExternalInput")

    x_t = din("x", (PADN, H), f32)
    idx_t = din("idx", (128, IDXW), i16)
    ids_t = din("ids", (128, NCHUNK), f32)
    odeg_t = din("odegT", (128, TILES), i32)
    ideg_t = din("idegT", (128, TILES), i32)
    ws_t = din("ws", (128, 3 * H), f32)
    bbc_t = din("bbc", (128, 3 * H), f32)
    wpt_t = din("wpt", (128, C), f32)
    bp_t = din("bp", (C, 1), f32)
    iota_t = din("iota", (128, 4 * 128), f32)
    ident_t = din("ident", (128, 128), f32)
    onesm_t = din("onesm", (128, 2), f32)

    cam_t = nc.dram_tensor("cam", [C, NPC], f32, kind="ExternalOutput")
    seg_t = nc.dram_tensor("seg", [C, 1], f32, kind="ExternalOutput")

    t_own = nc.dram_tensor("t_own", [PADN, H], f32)
    t_full = nc.dram_tensor("t_full", [GPAD, H], f32, addr_space="Shared")
    hg_in = nc.dram_tensor("hg_in", [128, 1], f32)
    hg_out = nc.dram_tensor("hg_out", [128, 1], f32, addr_space="Shared")

    rg = [list(range(N_CORES))]
    plan_by_sg = {}
    for sg, b, cc, cgb, ioff in call_plan:
        plan_by_sg.setdefault(sg, []).append((b, cc, cgb, ioff))

    CAM_BATCH = 14

    with tile.TileContext(nc) as tc:
        with (
            tc.tile_pool(name="const", bufs=1) as cpool,
            tc.tile_pool(name="g", bufs=8) as gpool,
            tc.tile_pool(name="oh", bufs=14) as ohpool,
            tc.tile_pool(name="stg", bufs=3) as stpool,
            tc.tile_pool(name="cams", bufs=2) as campool,
            tc.tile_pool(name="psS", bufs=3, space="PSUM") as psS,
            tc.tile_pool(name="psT", bufs=2, space="PSUM") as psT,
            tc.tile_pool(name="psH", bufs=1, space="PSUM") as psH,
            tc.tile_pool(name="psC", bufs=2, space="PSUM") as psC,
        ):
            # ---------------- constants & prologue ----------------
            h_sb = cpool.tile([128, TILES * TILE], f32)
            isb = cpool.tile([128, IDXW], i16)
            nc.sync.dma_start(out=isb[:], in_=idx_t[:, :])
            idsb = cpool.tile([128, NCHUNK], f32)
            nc.sync.dma_start(out=idsb[:], in_=ids_t[:, :])
            iotb = cpool.tile([128, 4 * 128], f32)
            nc.sync.dma_start(out=iotb[:], in_=iota_t[:, :])
            identb = cpool.tile([128, 128], f32)
            nc.sync.dma_start(out=identb[:], in_=ident_t[:, :])
            wsb = cpool.tile([128, 3 * H], f32)
            nc.sync.dma_start(out=wsb[:], in_=ws_t[:, :])
            bbcb = cpool.tile([128, 3 * H], f32)
            nc.sync.dma_start(out=bbcb[:], in_=bbc_t[:, :])
            wptb = cpool.tile([128, C], f32)
            nc.sync.dma_start(out=wptb[:], in_=wpt_t[:, :])
            bpb = cpool.tile([C, 1], f32)
            nc.sync.dma_start(out=bpb[:], in_=bp_t[:, :])
            onesb = cpool.tile([128, 2], f32)
            nc.sync.dma_start(out=onesb[:], in_=onesm_t[:, :])

            odegi = cpool.tile([128, TILES], i32)
            nc.sync.dma_start(out=odegi[:], in_=odeg_t[:, :])
            idegi = cpool.tile([128, TILES], i32)
            nc.sync.dma_start(out=idegi[:], in_=ideg_t[:, :])
            outn = cpool.tile([128, TILES], f32)
            innn = cpool.tile([128, TILES], f32)
            normio = cpool.tile([128, TILES], f32)
            tmpn = cpool.tile([128, TILES], f32)
            for deg, dnorm in ((odegi, outn), (idegi, innn)):
                nc.vector.tensor_copy(out=tmpn[:], in_=deg[:])
                nc.vector.tensor_scalar_max(tmpn[:], tmpn[:], 1.0)
                nc.vector.reciprocal(out=tmpn[:], in_=tmpn[:])
                nc.scalar.sqrt(out=dnorm[:], in_=tmpn[:])
            nc.vector.tensor_mul(out=normio[:], in0=outn[:], in1=innn[:])
            # sqrt(max(in_deg,1)) for the bias fold diag
            sqdeg = cpool.tile([128, TILES], f32)
            nc.vector.tensor_copy(out=tmpn[:], in_=idegi[:])
            nc.vector.tensor_scalar_max(tmpn[:], tmpn[:], 1.0)
            nc.scalar.sqrt(out=sqdeg[:], in_=tmpn[:])

            # x -> h (node-major tiles side by side), scaled by out_norm
            h3d = h_sb[:].rearrange("p (t f) -> p t f", f=H)
            nc.sync.dma_start(
                out=h3d,
                in_=x_t[:, :].rearrange("(t p) f -> p t f", p=128),
            )
            onap = outn[:]
            on_bcast = bass.AP(
                onap.tensor, onap.offset, [list(onap.ap[0]), [1, TILES], [0, H]]
            )
            nc.vector.tensor_tensor(
                out=h3d, in0=h3d, in1=on_bcast, op=mybir.AluOpType.mult
            )

            # ---------------- layers ----------------
            for layer in range(3):
                w_sl = wsb[:, layer * H : (layer + 1) * H]
                b_sl = bbcb[:, layer * H : (layer + 1) * H]

                for t in range(TILES):
                    hs_sl = h_sb[:, t * TILE : (t + 1) * TILE]
                    ps_tr = psT.tile([128, 128], f32, tag="pt")
                    nc.tensor.transpose(out=ps_tr[:], in_=hs_sl, identity=identb[:])
                    hsT = stpool.tile([128, 128], f32, tag="hsT")
                    nc.scalar.copy(out=hsT[:], in_=ps_tr[:])
                    ps_t = psT.tile([128, 128], f32, tag="pt")
                    nc.tensor.matmul(
                        out=ps_t[:], lhsT=hsT[:], rhs=w_sl, start=True, stop=True
                    )
                    tsb = stpool.tile([128, 128], f32, tag="tsb")
                    nc.scalar.copy(out=tsb[:], in_=ps_t[:])
                    nc.sync.dma_start(
                        out=t_own[t * TILE : (t + 1) * TILE, :], in_=tsb[:]
                    )

                nc.gpsimd.collective_compute(
                    "AllGather",
                    mybir.AluOpType.bypass,
                    replica_groups=rg,
                    ins=[t_own.ap().opt()],
                    outs=[t_full.ap().opt()],
                )

                for sg in range(N_SG):
                    g_tiles = {}
                    for b, cc, cgb, ioff in plan_by_sg[sg]:
                        if cc == 0:
                            continue
                        g = gpool.tile([128, cc, H], f32, tag="g")
                        nc.gpsimd.dma_gather(
                            g[:],
                            t_full[b * BUCKET : (b + 1) * BUCKET, :],
                            isb[:, ioff : ioff + cc * 8],
                            cc * TILE,
                            cc * TILE,
                            H,
                            single_packet=False,
                            queue_num=(sg + b) % 4,
                        )
                        g_tiles[b] = g

                    ts = range(sg * SG_TILES, min((sg + 1) * SG_TILES, TILES))
                    for t in ts:
                        # one-hot batches for this tile (4 chunks per DVE op)
                        ohb = int(tile_oh_base[t])
                        nch = len(tile_chunks[t])
                        batches = []
                        for j in range((nch + 3) // 4):
                            oh = ohpool.tile([128, 4, 128], f32, tag="oh")
                            c0 = ohb + 4 * j
                            iap = idsb[:, c0 : c0 + 1]
                            in0 = bass.AP(
                                iap.tensor,
                                iap.offset,
                                [list(iap.ap[0]), [1, 4], [0, 128]],
                            )
                            nc.vector.tensor_tensor(
                                out=oh[:],
                                in0=in0,
                                in1=iotb[:].rearrange("p (a b) -> p a b", a=4),
                                op=mybir.AluOpType.is_equal,
                            )
                            batches.append(oh)
                        ps = psS.tile([128, 128], f32, tag="ps")
                        for k, (b, slot) in enumerate(tile_chunks[t]):
                            nc.tensor.matmul(
                                out=ps[:],
                                lhsT=batches[k // 4][:, k % 4, :],
                                rhs=g_tiles[b][:, slot, :],
                                start=(k == 0),
                                stop=False,
                            )
                        # bias fold: += diag(sqrt(deg_in)) @ b_bcast
                        dg = ohpool.tile([128, 4, 128], f32, tag="oh")
                        nc.vector.tensor_tensor(
                            out=dg[:, 0, :],
                            in0=identb[:],
                            in1=sqdeg[:, t : t + 1].to_broadcast([128, 128]),
                            op=mybir.AluOpType.mult,
                        )
                        nc.tensor.matmul(
                            out=ps[:],
                            lhsT=dg[:, 0, :],
                            rhs=b_sl,
                            start=False,
                            stop=True,
                        )
                        scale = (
                            normio[:, t : t + 1] if layer < 2 else innn[:, t : t + 1]
                        )
                        nc.scalar.activation(
                            out=h_sb[:, t * TILE : (t + 1) * TILE],
                            in_=ps[:],
                            func=mybir.ActivationFunctionType.Relu,
                            scale=scale,
                        )

            # ---------------- final: mean-pool + CAM ----------------
            ps_hg = psH.tile([128, 1], f32, tag="hg")
            camstg = None
            for t in range(TILES):
                h3_sl = h_sb[:, t * TILE : (t + 1) * TILE]
                ones_col = onesb[:, 0:1] if t < TILES - 1 else onesb[:, 1:2]
                nc.tensor.matmul(
                    out=ps_hg[:],
                    lhsT=h3_sl,
                    rhs=ones_col,
                    start=(t == 0),
                    stop=(t == TILES - 1),
                )
                ps_tr = psT.tile([128, 128], f32, tag="pt")
                nc.tensor.transpose(out=ps_tr[:], in_=h3_sl, identity=identb[:])
                h3T = stpool.tile([128, 128], f32, tag="hsT")
                nc.scalar.copy(out=h3T[:], in_=ps_tr[:])
                ps_cam = psC.tile([C, 128], f32, tag="pc")
                nc.tensor.matmul(
                    out=ps_cam[:], lhsT=wptb[:], rhs=h3T[:], start=True, stop=True
                )
                bi = t % CAM_BATCH
                if bi == 0:
                    camstg = campool.tile([C, CAM_BATCH * TILE], f32, tag="cam")
                nc.scalar.copy(
                    out=camstg[:, bi * TILE : (bi + 1) * TILE], in_=ps_cam[:]
                )
                if bi == CAM_BATCH - 1 or t == TILES - 1:
                    c0 = (t - bi) * TILE
                    c1 = min((t + 1) * TILE, NPC)
                    nc.sync.dma_start(out=cam_t[:, c0:c1], in_=camstg[:, : c1 - c0])

            hgsb = stpool.tile([128, 1], f32, tag="hg1")
            nc.scalar.copy(out=hgsb[:], in_=ps_hg[:])
            nc.sync.dma_start(out=hg_in[:, :], in_=hgsb[:])
            nc.gpsimd.collective_compute(
                "AllReduce",
                mybir.AluOpType.add,
                replica_groups=rg,
                ins=[hg_in.ap().opt()],
                outs=[hg_out.ap().opt()],
            )
            hgm = stpool.tile([128, 1], f32, tag="hg2")
            nc.sync.dma_start(out=hgm[:], in_=hg_out[:, :])
            nc.vector.tensor_scalar_mul(hgm[:], hgm[:], 1.0 / N)
            ps_seg = psH.tile([C, 1], f32, tag="hg")
            nc.tensor.matmul(
                out=ps_seg[:], lhsT=wptb[:], rhs=hgm[:], start=True, stop=True
            )
            segsb = stpool.tile([C, 1], f32, tag="seg")
            nc.scalar.activation(
                out=segsb[:],
                in_=ps_seg[:],
                func=mybir.ActivationFunctionType.Identity,
                bias=bpb[:, 0:1],
            )
            nc.sync.dma_start(out=seg_t[:, :], in_=segsb[:])

    nc.compile()

    in_maps = []
    for c in range(N_CORES):
        in_maps.append(
            {
                "x": x_cores[c],
                "idx": idx_cores[c],
                "ids": ids_cores[c],
                "odegT": odeg_cores[c],
                "idegT": ideg_cores[c],
                "ws": ws_np,
                "bbc": bbc_np,
                "wpt": wpt_np,
                "bp": bp_np,
                "iota": iota_np,
                "ident": ident_np,
                "onesm": onesm_np,
            }
        )

    trace = os.environ.get("GCN_TRACE", "0") == "1"
    res = run_bass_kernel_spmd(
        nc, in_maps, core_ids=list(range(N_CORES)), trace=trace
    )
    kernel.last_result = res

    cam_full = np.concatenate(
        [res.results[c]["cam"] for c in range(N_CORES)], axis=1
    )
    seg_out = res.results[0]["seg"].reshape(1, C)
    return seg_out, cam_full


# revision 7
# speedup vs baseline: 1.4914x; 1.4914x over previous
"""Trainium2 Bass kernel for BasicGCNSegmentation (3-layer GCN + CAM head).

Strategy (8 NeuronCores, SPMD):
  - Nodes sharded contiguously: core c owns rows [c*12500, (c+1)*12500),
    padded on host to 12544 = 98*128 per core (100352 global padded rows).
  - Per layer: each core transforms its own node slice t = (h .* out_norm) @ W
    (PE, via per-tile transpose), AllGather of t -> replicated table in DRAM,
    then per-edge row gather (dma_gather, int16 indices in 4 address buckets,
    spread over 4 SWDGE queues) + segment-sum via one-hot matmuls accumulating
    in PSUM, epilogue relu with folded degree norms (ACT).
  - deg-norm bias fold: relu(in_n*S + b) = relu(norm*(S + b*sqrt(deg_in)))
    with norm = in_n (*out_n for layers that feed the next transform), so the
    bias enters as one rank-1 matmul into PSUM and the epilogue is a single
    scaled relu.
  - Final: node-mean via ones-matmul + AllReduce; CAM = Wp @ h3^T per tile.

Host does graph restructuring only (sharding, sorting, padding, degree
counting, int16 index marshalling); all floating-point math runs on device.
"""

import os

import numpy as np

import concourse.bacc as bacc
import concourse.bass as bass
import concourse.mybir as mybir
import concourse.tile as tile
from concourse.bass_utils import run_bass_kernel_spmd

N_CORES = 8
N = 100000
E = 1600000
H = 128
C = 40
NPC = N // N_CORES          # 12500 real nodes per core
TILES = 98
TILE = 128
PADN = TILES * TILE          # 12544 padded nodes per core
GPAD = PADN * N_CORES        # 100352 padded global rows
N_BUCKETS = 4
BUCKET = GPAD // N_BUCKETS   # 25088 rows per gather-address bucket (<32768)
SG_TILES = 3                 # dst tiles per gather supergroup
N_SG = (TILES + SG_TILES - 1) // SG_TILES  # 33 (last has 2 tiles)

f32 = mybir.dt.float32
bf16 = mybir.dt.bfloat16
i32 = mybir.dt.int32
i16 = mybir.dt.int16


def _prep_graph(src, dst):
    """Shard/sort edges; build per-core int16 gather indices + one-hot ids.

    Chunk structure (counts per (tile, bucket)) is the max over cores, so one
    SPMD program fits all cores; per-core arrays differ only in values.
    """
    src = src.astype(np.int64)
    dst = dst.astype(np.int64)
    src_pad = src + (src // NPC) * (PADN - NPC)

    core_of = dst // NPC
    dl = dst % NPC
    t_id = dl >> 7
    wid = (dl & 127).astype(np.int32)
    b_id = src_pad // BUCKET
    rel = (src_pad - b_id * BUCKET).astype(np.int16)

    key = (core_of * TILES * N_BUCKETS + t_id * N_BUCKETS + b_id).astype(np.int64)
    counts = np.bincount(key, minlength=N_CORES * TILES * N_BUCKETS).reshape(
        N_CORES, TILES, N_BUCKETS
    )
    chunks_tb = -(-counts.max(axis=0) // TILE)  # [98, 4] ceil
    empty = chunks_tb.sum(axis=1) == 0
    chunks_tb[empty, 0] = 1
    chunks_tb = chunks_tb.astype(np.int64)

    # global chunk order: (sg, b, t within sg, k); calls are (sg, b) blocks
    cg_start = np.zeros((TILES, N_BUCKETS), dtype=np.int64)
    call_plan = []  # (sg, b, n_chunks, cg_base, idx_col_off)
    cg = 0
    idx_off = 0
    for sg in range(N_SG):
        ts = range(sg * SG_TILES, min((sg + 1) * SG_TILES, TILES))
        for b in range(N_BUCKETS):
            cc = 0
            for t in ts:
                cg_start[t, b] = cg + cc
                cc += chunks_tb[t, b]
            call_plan.append((sg, b, int(cc), cg, idx_off))
            cg += cc
            idx_off += cc * 8  # int16 columns per chunk = 128/16
    nchunk = cg

    # per-tile consumption map: (bucket, slot within call); one-hot columns are
    # numbered per-tile-contiguous (sg, t, b, k), padded to x4 per tile
    call_base = {(sg, b): cgb for sg, b, cc, cgb, _ in call_plan}
    tile_chunks = []
    tile_oh_base = np.zeros(TILES, dtype=np.int64)
    ohc = 0
    for t in range(TILES):
        sg = t // SG_TILES
        tile_oh_base[t] = ohc
        lst = []
        for b in range(N_BUCKETS):
            for k in range(chunks_tb[t, b]):
                cgk = int(cg_start[t, b] + k)
                lst.append((b, cgk - call_base[(sg, b)]))
        tile_chunks.append(lst)
        ohc += (len(lst) + 3) // 4 * 4
    nchunk_oh = ohc

    idx_cores, ids_cores = [], []
    for c in range(N_CORES):
        m = core_of == c
        tc_, bc_, relc, widc = t_id[m], b_id[m], rel[m], wid[m]
        order = np.lexsort((bc_, tc_))
        tc_, bc_, relc, widc = tc_[order], bc_[order], relc[order], widc[order]
        gkey = tc_ * N_BUCKETS + bc_
        grp_counts = np.bincount(gkey, minlength=TILES * N_BUCKETS)
        grp_first = np.concatenate([[0], np.cumsum(grp_counts)[:-1]])
        within = np.arange(len(gkey)) - grp_first[gkey]
        pos = cg_start[tc_, bc_] * TILE + within

        # one-hot position: per-tile-contiguous numbering
        boff = np.zeros((TILES, N_BUCKETS), dtype=np.int64)
        for t in range(TILES):
            o = 0
            for b in range(N_BUCKETS):
                boff[t, b] = o
                o += chunks_tb[t, b]
        ohpos = (tile_oh_base[tc_] + boff[tc_, bc_]) * TILE + within

        idx_flat = np.zeros(nchunk * TILE, dtype=np.int16)
        ids_flat = np.full(nchunk_oh * TILE, -1.0, dtype=np.float32)
        idx_flat[pos] = relc
        ids_flat[ohpos] = widc

        cols = []
        for sg, b, cc, cgb, _ in call_plan:
            if cc == 0:
                continue
            blk = idx_flat[cgb * TILE : (cgb + cc) * TILE]
            cols.append(np.tile(blk.reshape(-1, 16).T, (8, 1)))
        import ml_dtypes

        idx_cores.append(np.ascontiguousarray(np.concatenate(cols, axis=1)))
        ids_cores.append(
            np.ascontiguousarray(
                ids_flat.reshape(nchunk_oh, TILE).T.astype(ml_dtypes.bfloat16)
            )
        )

    return (
        chunks_tb,
        nchunk_oh,
        call_plan,
        tile_chunks,
        tile_oh_base,
        idx_cores,
        ids_cores,
    )


def kernel(features, W1, b1, W2, b2, W3, b3, Wp, bp, src, dst):
    assert features.shape == (N, H) and src.shape == (E,) and dst.shape == (E,)

    out_deg = np.bincount(src, minlength=N).astype(np.int32)
    in_deg = np.bincount(dst, minlength=N).astype(np.int32)

    (
        chunks_tb,
        NCHUNK,
        call_plan,
        tile_chunks,
        tile_oh_base,
        idx_cores,
        ids_cores,
    ) = _prep_graph(src, dst)
    IDXW = sum(cc * 8 for _, _, cc, _, _ in call_plan)

    x_cores, odeg_cores, ideg_cores = [], [], []
    for c in range(N_CORES):
        xs = np.zeros((PADN, H), dtype=np.float32)
        xs[:NPC] = np.asarray(features)[c * NPC : (c + 1) * NPC]
        x_cores.append(xs)
        od = np.zeros(PADN, dtype=np.int32)
        od[:NPC] = out_deg[c * NPC : (c + 1) * NPC]
        idg = np.zeros(PADN, dtype=np.int32)
        idg[:NPC] = in_deg[c * NPC : (c + 1) * NPC]
        odeg_cores.append(np.ascontiguousarray(od.reshape(TILES, TILE).T))
        ideg_cores.append(np.ascontiguousarray(idg.reshape(TILES, TILE).T))

    import ml_dtypes

    iota_np = np.ascontiguousarray(
        np.tile(
            np.broadcast_to(np.arange(TILE, dtype=np.float32), (TILE, TILE)), (1, 4)
        ).astype(ml_dtypes.bfloat16)
    )
    ident_np = np.eye(TILE, dtype=np.float32)
    onesm_np = np.zeros((TILE, 2), dtype=np.float32)
    onesm_np[:, 0] = 1.0
    onesm_np[: NPC - 97 * TILE, 1] = 1.0
    bbc_np = np.ascontiguousarray(
        np.broadcast_to(
            np.concatenate([b1, b2, b3]).astype(np.float32), (TILE, 3 * H)
        ).astype(ml_dtypes.bfloat16)
    )
    ws_np = np.ascontiguousarray(np.concatenate([W1, W2, W3], axis=1).astype(np.float32))
    wpt_np = np.ascontiguousarray(np.asarray(Wp).astype(np.float32).T)
    bp_np = np.asarray(bp).astype(np.float32).reshape(C, 1)

    nc = bacc.Bacc(
        "TRN2",
        target_bir_lowering=False,
        debug=False,
        num_devices=N_CORES,
        num_swdge_queues=4,
    )

    def din(name, shape, dtype):
        return nc.dram_tensor(name, list(shape), dtype, kind="ExternalInput")

    x_t = din("x", (PADN, H), f32)
    idx_t = din("idx", (128, IDXW), i16)
    ids_t = din("ids", (128, NCHUNK), bf16)
    odeg_t = din("odegT", (128, TILES), i32)
    ideg_t = din("idegT", (128, TILES), i32)
    ws_t = din("ws", (128, 3 * H), f32)
    bbc_t = din("bbc", (128, 3 * H), bf16)
    wpt_t = din("wpt", (128, C), f32)
    bp_t = din("bp", (C, 1), f32)
    iota_t = din("iota", (128, 4 * 128), bf16)
    ident_t = din("ident", (128, 128), f32)
    onesm_t = din("onesm", (128, 2), f32)

    cam_t = nc.dram_tensor("cam", [C, NPC], f32, kind="ExternalOutput")
    seg_t = nc.dram_tensor("seg", [C, 1], f32, kind="ExternalOutput")

    t_own = nc.dram_tensor("t_own", [PADN, H], bf16)
    t_full = nc.dram_tensor("t_full", [GPAD, H], bf16, addr_space="Shared")
    hg_in = nc.dram_tensor("hg_in", [128, 1], f32)
    hg_out = nc.dram_tensor("hg_out", [128, 1], f32, addr_space="Shared")

    rg = [list(range(N_CORES))]
    plan_by_sg = {}
    for sg, b, cc, cgb, ioff in call_plan:
        plan_by_sg.setdefault(sg, []).append((b, cc, cgb, ioff))

    CAM_BATCH = 14

    with tile.TileContext(nc) as tc:
        with (
            tc.tile_pool(name="const", bufs=1) as cpool,
            tc.tile_pool(name="g", bufs=8) as gpool,
            tc.tile_pool(name="oh", bufs=14) as ohpool,
            tc.tile_pool(name="stg", bufs=3) as stpool,
            tc.tile_pool(name="cams", bufs=2) as campool,
            tc.tile_pool(name="psS", bufs=3, space="PSUM") as psS,
            tc.tile_pool(name="psT", bufs=2, space="PSUM") as psT,
            tc.tile_pool(name="psH", bufs=1, space="PSUM") as psH,
            tc.tile_pool(name="psC", bufs=2, space="PSUM") as psC,
        ):
            # ---------------- constants & prologue ----------------
            h_sb = cpool.tile([128, TILES * TILE], f32)
            isb = cpool.tile([128, IDXW], i16)
            nc.sync.dma_start(out=isb[:], in_=idx_t[:, :])
            idsb = cpool.tile([128, NCHUNK], bf16)
            nc.sync.dma_start(out=idsb[:], in_=ids_t[:, :])
            iotb = cpool.tile([128, 4 * 128], bf16)
            nc.sync.dma_start(out=iotb[:], in_=iota_t[:, :])
            identb = cpool.tile([128, 128], f32)
            nc.sync.dma_start(out=identb[:], in_=ident_t[:, :])
            wsb = cpool.tile([128, 3 * H], f32)
            nc.sync.dma_start(out=wsb[:], in_=ws_t[:, :])
            bbcb = cpool.tile([128, 3 * H], bf16)
            nc.sync.dma_start(out=bbcb[:], in_=bbc_t[:, :])
            wptb = cpool.tile([128, C], f32)
            nc.sync.dma_start(out=wptb[:], in_=wpt_t[:, :])
            bpb = cpool.tile([C, 1], f32)
            nc.sync.dma_start(out=bpb[:], in_=bp_t[:, :])
            onesb = cpool.tile([128, 2], f32)
            nc.sync.dma_start(out=onesb[:], in_=onesm_t[:, :])

            odegi = cpool.tile([128, TILES], i32)
            nc.sync.dma_start(out=odegi[:], in_=odeg_t[:, :])
            idegi = cpool.tile([128, TILES], i32)
            nc.sync.dma_start(out=idegi[:], in_=ideg_t[:, :])
            outn = cpool.tile([128, TILES], f32)
            innn = cpool.tile([128, TILES], f32)
            normio = cpool.tile([128, TILES], f32)
            tmpn = cpool.tile([128, TILES], f32)
            for deg, dnorm in ((odegi, outn), (idegi, innn)):
                nc.vector.tensor_copy(out=tmpn[:], in_=deg[:])
                nc.vector.tensor_scalar_max(tmpn[:], tmpn[:], 1.0)
                nc.vector.reciprocal(out=tmpn[:], in_=tmpn[:])
                nc.scalar.sqrt(out=dnorm[:], in_=tmpn[:])
            nc.vector.tensor_mul(out=normio[:], in0=outn[:], in1=innn[:])
            # sqrt(max(in_deg,1)) for the bias fold diag (bf16 operands)
            sqdeg = cpool.tile([128, TILES], f32)
            nc.vector.tensor_copy(out=tmpn[:], in_=idegi[:])
            nc.vector.tensor_scalar_max(tmpn[:], tmpn[:], 1.0)
            nc.scalar.sqrt(out=sqdeg[:], in_=tmpn[:])
            sqdegb = cpool.tile([128, TILES], bf16)
            nc.vector.tensor_copy(out=sqdegb[:], in_=sqdeg[:])
            identbf = cpool.tile([128, 128], bf16)
            nc.vector.tensor_copy(out=identbf[:], in_=identb[:])

            # x -> h (node-major tiles side by side), scaled by out_norm
            h3d = h_sb[:].rearrange("p (t f) -> p t f", f=H)
            nc.sync.dma_start(
                out=h3d,
                in_=x_t[:, :].rearrange("(t p) f -> p t f", p=128),
            )
            onap = outn[:]
            on_bcast = bass.AP(
                onap.tensor, onap.offset, [list(onap.ap[0]), [1, TILES], [0, H]]
            )
            nc.vector.tensor_tensor(
                out=h3d, in0=h3d, in1=on_bcast, op=mybir.AluOpType.mult
            )

            # ---------------- layers ----------------
            for layer in range(3):
                w_sl = wsb[:, layer * H : (layer + 1) * H]
                b_sl = bbcb[:, layer * H : (layer + 1) * H]

                for t in range(TILES):
                    hs_sl = h_sb[:, t * TILE : (t + 1) * TILE]
                    ps_tr = psT.tile([128, 128], f32, tag="pt")
                    nc.tensor.transpose(out=ps_tr[:], in_=hs_sl, identity=identb[:])
                    hsT = stpool.tile([128, 128], f32, tag="hsT")
                    nc.scalar.copy(out=hsT[:], in_=ps_tr[:])
                    ps_t = psT.tile([128, 128], f32, tag="pt")
                    nc.tensor.matmul(
                        out=ps_t[:], lhsT=hsT[:], rhs=w_sl, start=True, stop=True
                    )
                    tsb = stpool.tile([128, 128], bf16, tag="tsb")
                    nc.scalar.copy(out=tsb[:], in_=ps_t[:])
                    nc.sync.dma_start(
                        out=t_own[t * TILE : (t + 1) * TILE, :], in_=tsb[:]
                    )

                nc.gpsimd.collective_compute(
                    "AllGather",
                    mybir.AluOpType.bypass,
                    replica_groups=rg,
                    ins=[t_own.ap().opt()],
                    outs=[t_full.ap().opt()],
                )

                for sg in range(N_SG):
                    g_tiles = {}
                    for b, cc, cgb, ioff in plan_by_sg[sg]:
                        if cc == 0:
                            continue
                        g = gpool.tile([128, cc, H], bf16, tag="g")
                        nc.gpsimd.dma_gather(
                            g[:],
                            t_full[b * BUCKET : (b + 1) * BUCKET, :],
                            isb[:, ioff : ioff + cc * 8],
                            cc * TILE,
                            cc * TILE,
                            H,
                            single_packet=False,
                            queue_num=(sg + b) % 4,
                        )
                        g_tiles[b] = g

                    ts = range(sg * SG_TILES, min((sg + 1) * SG_TILES, TILES))
                    for t in ts:
                        # one-hot batches for this tile (4 chunks per DVE op)
                        ohb = int(tile_oh_base[t])
                        nch = len(tile_chunks[t])
                        batches = []
                        for j in range((nch + 3) // 4):
                            oh = ohpool.tile([128, 4, 128], bf16, tag="oh")
                            c0 = ohb + 4 * j
                            iap = idsb[:, c0 : c0 + 1]
                            in0 = bass.AP(
                                iap.tensor,
                                iap.offset,
                                [list(iap.ap[0]), [1, 4], [0, 128]],
                            )
                            nc.vector.tensor_tensor(
                                out=oh[:],
                                in0=in0,
                                in1=iotb[:].rearrange("p (a b) -> p a b", a=4),
                                op=mybir.AluOpType.is_equal,
                            )
                            batches.append(oh)
                        ps = psS.tile([128, 128], f32, tag="ps")
                        for k, (b, slot) in enumerate(tile_chunks[t]):
                            nc.tensor.matmul(
                                out=ps[:],
                                lhsT=batches[k // 4][:, k % 4, :],
                                rhs=g_tiles[b][:, slot, :],
                                start=(k == 0),
                                stop=False,
                            )
                        # bias fold: += diag(sqrt(deg_in)) @ b_bcast
                        dg = ohpool.tile([128, 4, 128], bf16, tag="oh")
                        nc.vector.tensor_tensor(
                            out=dg[:, 0, :],
                            in0=identbf[:],
                            in1=sqdegb[:, t : t + 1].to_broadcast([128, 128]),
                            op=mybir.AluOpType.mult,
                        )
                        nc.tensor.matmul(
                            out=ps[:],
                            lhsT=dg[:, 0, :],
                            rhs=b_sl,
                            start=False,
                            stop=True,
                        )
                        scale = (
                            normio[:, t : t + 1] if layer < 2 else innn[:, t : t + 1]
                        )
                        nc.scalar.activation(
                            out=h_sb[:, t * TILE : (t + 1) * TILE],
                            in_=ps[:],
                            func=mybir.ActivationFunctionType.Relu,
                            scale=scale,
                        )

            # ---------------- final: mean-pool + CAM ----------------
            ps_hg = psH.tile([128, 1], f32, tag="hg")
            camstg = None
            for t in range(TILES):
                h3_sl = h_sb[:, t * TILE : (t + 1) * TILE]
                ones_col = onesb[:, 0:1] if t < TILES - 1 else onesb[:, 1:2]
                nc.tensor.matmul(
                    out=ps_hg[:],
                    lhsT=h3_sl,
                    rhs=ones_col,
                    start=(t == 0),
                    stop=(t == TILES - 1),
                )
                ps_tr = psT.tile([128, 128], f32, tag="pt")
                nc.tensor.transpose(out=ps_tr[:], in_=h3_sl, identity=identb[:])
                h3T = stpool.tile([128, 128], f32, tag="hsT")
                nc.scalar.copy(out=h3T[:], in_=ps_tr[:])
                ps_cam = psC.tile([C, 128], f32, tag="pc")
                nc.tensor.matmul(
                    out=ps_cam[:], lhsT=wptb[:], rhs=h3T[:], start=True, stop=True
                )
                bi = t % CAM_BATCH
                if bi == 0:
                    camstg = campool.tile([C, CAM_BATCH * TILE], f32, tag="cam")
                nc.scalar.copy(
                    out=camstg[:, bi * TILE : (bi + 1) * TILE], in_=ps_cam[:]
                )
                if bi == CAM_BATCH - 1 or t == TILES - 1:
                    c0 = (t - bi) * TILE
                    c1 = min((t + 1) * TILE, NPC)
                    nc.sync.dma_start(out=cam_t[:, c0:c1], in_=camstg[:, : c1 - c0])

            hgsb = stpool.tile([128, 1], f32, tag="hg1")
            nc.scalar.copy(out=hgsb[:], in_=ps_hg[:])
            nc.sync.dma_start(out=hg_in[:, :], in_=hgsb[:])
            nc.gpsimd.collective_compute(
                "AllReduce",
                mybir.AluOpType.add,
                replica_groups=rg,
                ins=[hg_in.ap().opt()],
                outs=[hg_out.ap().opt()],
            )
            hgm = stpool.tile([128, 1], f32, tag="hg2")
            nc.sync.dma_start(out=hgm[:], in_=hg_out[:, :])
            nc.vector.tensor_scalar_mul(hgm[:], hgm[:], 1.0 / N)
            ps_seg = psH.tile([C, 1], f32, tag="hg")
            nc.tensor.matmul(
                out=ps_seg[:], lhsT=wptb[:], rhs=hgm[:], start=True, stop=True
            )
            segsb = stpool.tile([C, 1], f32, tag="seg")
            nc.scalar.activation(
                out=segsb[:],
                in_=ps_seg[:],
                func=mybir.ActivationFunctionType.Identity,
                bias=bpb[:, 0:1],
            )
            nc.sync.dma_start(out=seg_t[:, :], in_=segsb[:])

    nc.compile()

    in_maps = []
    for c in range(N_CORES):
        in_maps.append(
            {
                "x": x_cores[c],
                "idx": idx_cores[c],
                "ids": ids_cores[c],
                "odegT": odeg_cores[c],
                "idegT": ideg_cores[c],
                "ws": ws_np,
                "bbc": bbc_np,
                "wpt": wpt_np,
                "bp": bp_np,
                "iota": iota_np,
                "ident": ident_np,
                "onesm": onesm_np,
            }
        )

    trace = os.environ.get("GCN_TRACE", "0") == "1"
    res = run_bass_kernel_spmd(
        nc, in_maps, core_ids=list(range(N_CORES)), trace=trace
    )
    kernel.last_result = res

    cam_full = np.concatenate(
        [res.results[c]["cam"] for c in range(N_CORES)], axis=1
    )
    seg_out = res.results[0]["seg"].reshape(1, C)
    return seg_out, cam_full
